# revision 8
# baseline (speedup 1.0000x reference)
"""Trainium2 Bass kernel for single-head causal attention.

Problem: B=8, T=2048, C=1024, HS=64
  q = x_q @ Wq; k = x_kv @ Wk; v = x_kv @ Wv        (all [B,T,HS])
  wei = softmax(mask(q @ k.T * C**-0.5))            ([B,T,T], causal)
  out = wei @ v                                      ([B,T,HS])

Sharding: data-parallel over batch B across 8 cores (1 batch element/core).

Per-core design (all in "transposed" layout so the softmax denominator
comes out of the PE for free):
  - host pre-transposes x to [C, T] slices so the contraction dim C lands
    on SBUF partitions (PE contracts over the partition dim).
  - projections: qT[64,T] = Wq.T @ x_q   (lhsT=Wq chunk, moving=xT chunk)
                 kvT[128,T] = [Wk|Wv].T @ x_kv  (full 128-wide array)
  - scores transposed:  ST[tk,tq] = kT_blk.T? -> matmul(lhsT=kT[64,128],
                 rhs=qT[64,512]) per (tk-tile 128, tq-tile 512), causal
                 blocks only (40 of 64).
  - softmax without max-subtraction (scores ~N(0,0.25^2): exp is safe):
                 P = exp(S/32) * diag-mask ; denominator handled by an
                 appended ones-column in V.
  - PV: outT'[65,tq] = sum_j matmul(lhsT=v'[tk 128,65], rhs=P[tk,tq 512]);
                 row 64 = denominator.
  - finalize: PE-transpose [65,128] blocks -> [128,65], reciprocal of
                 col 64, per-partition broadcast multiply, DMA out.
"""

import sys

sys.path.insert(0, "/opt/trn_rl_repo")

import numpy as np

import concourse.bass as bass
from concourse import bacc
import concourse.mybir as mybir
import concourse.tile as tile
from concourse.bass_utils import run_bass_kernel_spmd
from concourse.masks import make_identity

FP32 = mybir.dt.float32
T, C, HS = 2048, 1024, 64
NSLICE = 4          # tq slices of 512
TS = T // NSLICE    # 512
CK = C // 128       # 8 c-chunks
NJ = T // 128       # 16 tk tiles of 128
SCALE = float(C) ** -0.5

# matmul dtype knob: float32 (exact, 4 cyc/row) vs float32r (1 cyc/row @ N>=512)
MM_DT = mybir.dt.float32r


DT = MM_DT  # dtype for every tensor feeding a matmul


def build_bass():
    nc = bacc.Bacc(None, target_bir_lowering=False)
    xq = nc.dram_tensor("xq", [NSLICE, C, TS], DT, kind="ExternalInput").ap()
    xk = nc.dram_tensor("xk", [NSLICE, C, TS], DT, kind="ExternalInput").ap()
    wq = nc.dram_tensor("wq", [C, HS], DT, kind="ExternalInput").ap()
    wkv = nc.dram_tensor("wkv", [C, 2 * HS], DT, kind="ExternalInput").ap()
    out = nc.dram_tensor("out", [T, HS], FP32, kind="ExternalOutput").ap()

    with tile.TileContext(nc) as tc:
        with (
            tc.tile_pool(name="singles", bufs=1) as singles,
            tc.tile_pool(name="xpool", bufs=2) as xpool,
            tc.tile_pool(name="projsb", bufs=1) as projsb,
            tc.tile_pool(name="pblk", bufs=2) as pblk,
            tc.tile_pool(name="fin", bufs=4) as fin,
            tc.tile_pool(name="pp_q", bufs=2, space="PSUM") as pp_q,
            tc.tile_pool(name="pp_kv", bufs=2, space="PSUM") as pp_kv,
            tc.tile_pool(name="pp_st", bufs=2, space="PSUM") as pp_st,
            tc.tile_pool(name="pp_o", bufs=1, space="PSUM") as pp_o,
            tc.tile_pool(name="pp_tr", bufs=1, space="PSUM") as pp_tr,
        ):
            # ---- one-time constants ----
            wq_sb = singles.tile([128, CK, HS], DT)
            nc.sync.dma_start(out=wq_sb, in_=wq.rearrange("(c p) h -> p c h", p=128))
            wkv_sb = singles.tile([128, CK, 2 * HS], DT)
            nc.sync.dma_start(out=wkv_sb, in_=wkv.rearrange("(c p) h -> p c h", p=128))

            ident = singles.tile([128, 128], FP32)
            make_identity(nc, ident)

            # diag masks: mask[m][x, y] = 1.0 if y - x >= 128*m else 0.0
            masks = singles.tile([128, 4, TS], FP32)
            nc.gpsimd.memset(masks, 1.0)
            for m in range(4):
                nc.gpsimd.affine_select(
                    out=masks[:, m, :],
                    in_=masks[:, m, :],
                    compare_op=mybir.AluOpType.is_ge,
                    fill=0.0,
                    base=-128 * m,
                    pattern=[[1, TS]],
                    channel_multiplier=-1,
                )

            # persistent activations
            qT_sb = projsb.tile([64, T], DT)        # qT
            kvT_sb = projsb.tile([128, T], DT)      # rows 0:64 kT, 64:128 vT
            vT_sb = projsb.tile([64, T], FP32)      # vT shifted to base 0
            v_sb = projsb.tile([128, NJ, HS + 1], DT)  # v natural + ones col
            ones_f32 = singles.tile([128, 1], FP32)
            nc.vector.memset(ones_f32, 1.0)
            nc.vector.tensor_copy(
                v_sb[:, :, HS : HS + 1], ones_f32.broadcast_to((128, NJ, 1))
            )

            for s in range(NSLICE):
                t0 = s * TS
                # ---- stream x slice ----
                xq_t = xpool.tile([128, CK, TS], DT, tag="xq")
                nc.sync.dma_start(
                    out=xq_t, in_=xq[s].rearrange("(c p) t -> p c t", p=128)
                )
                xk_t = xpool.tile([128, CK, TS], DT, tag="xk")
                nc.sync.dma_start(
                    out=xk_t, in_=xk[s].rearrange("(c p) t -> p c t", p=128)
                )

                # ---- projections for this slice ----
                q_ps = pp_q.tile([64, TS], FP32)
                kv_ps = pp_kv.tile([128, TS], FP32)
                for ci in range(CK):
                    nc.tensor.matmul(
                        q_ps,
                        wq_sb[:, ci, :],
                        xq_t[:, ci, :],
                        start=(ci == 0),
                        stop=(ci == CK - 1),
                    )
                    nc.tensor.matmul(
                        kv_ps,
                        wkv_sb[:, ci, :],
                        xk_t[:, ci, :],
                        start=(ci == 0),
                        stop=(ci == CK - 1),
                    )
                nc.scalar.copy(qT_sb[:, t0 : t0 + TS], q_ps)
                nc.scalar.copy(kvT_sb[:, t0 : t0 + TS], kv_ps)
                # shift vT rows 64:128 -> base 0 (DMA can cross partitions)
                nc.sync.dma_start(
                    out=vT_sb[:, t0 : t0 + TS],
                    in_=kvT_sb[64:128, t0 : t0 + TS].bitcast(FP32),
                )
                # v natural layout via PE transpose of [64,128] blocks
                for jj in range(4):
                    j = 4 * s + jj
                    v_ps = pp_tr.tile([128, HS], FP32, tag="tr")
                    nc.tensor.transpose(
                        v_ps,
                        vT_sb[:, j * 128 : (j + 1) * 128],
                        ident[0:64, 0:64],
                    )
                    nc.vector.tensor_copy(v_sb[:, j, 0:HS], v_ps)

                # ---- attention for tq-tile i = s ----
                i = s
                nj = 4 * i + 4
                p_all = pblk.tile([128, NJ, TS], DT, tag="p")
                for j in range(nj):
                    st_ps = pp_st.tile([128, TS], FP32)
                    nc.tensor.matmul(
                        st_ps,
                        kvT_sb[0:64, j * 128 : (j + 1) * 128],
                        qT_sb[:, t0 : t0 + TS],
                        start=True,
                        stop=True,
                    )
                    nc.scalar.activation(
                        out=p_all[:, j, :],
                        in_=st_ps,
                        func=mybir.ActivationFunctionType.Exp,
                        scale=SCALE,
                    )
                    if j >= 4 * i:
                        nc.vector.tensor_mul(
                            p_all[:, j, :], p_all[:, j, :], masks[:, j - 4 * i, :].bitcast(DT)
                        )
                o_ps = pp_o.tile([HS + 1, TS], FP32)
                for j in range(nj):
                    nc.tensor.matmul(
                        o_ps,
                        v_sb[:, j, :],
                        p_all[:, j, :],
                        start=(j == 0),
                        stop=(j == nj - 1),
                    )
                ot_sb = fin.tile([HS + 1, TS], FP32, tag="ot")
                nc.scalar.copy(ot_sb, o_ps)
                for u in range(4):
                    tr_ps = pp_tr.tile([128, HS + 1], FP32, tag="tr")
                    nc.tensor.transpose(
                        tr_ps,
                        ot_sb[:, u * 128 : (u + 1) * 128],
                        ident[0 : HS + 1, 0 : HS + 1],
                    )
                    rec = fin.tile([128, 1], FP32, tag="rec")
                    nc.vector.reciprocal(rec, tr_ps[:, HS : HS + 1])
                    ob = fin.tile([128, HS], FP32, tag="ob")
                    nc.vector.tensor_scalar_mul(ob, tr_ps[:, 0:HS], rec)
                    r0 = (4 * i + u) * 128
                    nc.sync.dma_start(out=out[r0 : r0 + 128, :], in_=ob)
    nc.compile()
    return nc


_NC_CACHE = {}


def _get_nc():
    key = str(MM_DT)
    if key not in _NC_CACHE:
        _NC_CACHE[key] = build_bass()
    return _NC_CACHE[key]


def kernel(x_q, x_kv, Wq, Wk, Wv, _trace=False):
    B = x_q.shape[0]
    assert B == 8 and x_q.shape == (8, T, C)
    wkv = np.ascontiguousarray(np.concatenate([Wk, Wv], axis=1), dtype=np.float32)
    wq = np.ascontiguousarray(Wq, dtype=np.float32)
    # [B, T, C] -> [B, C, T] -> [B, NSLICE, C, TS] (t-slice major, contiguous)
    xqT = np.ascontiguousarray(
        x_q.transpose(0, 2, 1).reshape(B, C, NSLICE, TS).transpose(0, 2, 1, 3)
    ).astype(np.float32)
    xkT = np.ascontiguousarray(
        x_kv.transpose(0, 2, 1).reshape(B, C, NSLICE, TS).transpose(0, 2, 1, 3)
    ).astype(np.float32)

    in_maps = [
        {"xq": xqT[b], "xk": xkT[b], "wq": wq, "wkv": wkv} for b in range(B)
    ]
    nc = _get_nc()
    res = run_bass_kernel_spmd(nc, in_maps, core_ids=list(range(B)), trace=_trace)
    out = np.stack([r["out"] for r in res.results])
    if _trace:
        kernel.last_result = res
    return out


# revision 10
# speedup vs baseline: 1.5373x; 1.5373x over previous
"""Trainium2 Bass kernel for single-head causal attention.

Problem: B=8, T=2048, C=1024, HS=64
  q = x_q @ Wq; k = x_kv @ Wk; v = x_kv @ Wv        (all [B,T,HS])
  wei = softmax(mask(q @ k.T * C**-0.5))            ([B,T,T], causal)
  out = wei @ v                                      ([B,T,HS])

Sharding: data-parallel over batch B across 8 cores (1 batch element/core).

Per-core design (all in "transposed" layout so the softmax denominator
comes out of the PE for free):
  - host pre-transposes x to [C, T] slices so the contraction dim C lands
    on SBUF partitions (PE contracts over the partition dim).
  - projections: qT[64,T] = Wq.T @ x_q   (lhsT=Wq chunk, moving=xT chunk)
                 kvT[128,T] = [Wk|Wv].T @ x_kv  (full 128-wide array)
  - scores transposed:  ST[tk,tq] = kT_blk.T? -> matmul(lhsT=kT[64,128],
                 rhs=qT[64,512]) per (tk-tile 128, tq-tile 512), causal
                 blocks only (40 of 64).
  - softmax without max-subtraction (scores ~N(0,0.25^2): exp is safe):
                 P = exp(S/32) * diag-mask ; denominator handled by an
                 appended ones-column in V.
  - PV: outT'[65,tq] = sum_j matmul(lhsT=v'[tk 128,65], rhs=P[tk,tq 512]);
                 row 64 = denominator.
  - finalize: PE-transpose [65,128] blocks -> [128,65], reciprocal of
                 col 64, per-partition broadcast multiply, DMA out.
"""

import sys

sys.path.insert(0, "/opt/trn_rl_repo")

import numpy as np
import ml_dtypes

import concourse.bass as bass
from concourse import bacc
import concourse.mybir as mybir
import concourse.tile as tile
from concourse.bass_utils import run_bass_kernel_spmd
from concourse.masks import make_identity

FP32 = mybir.dt.float32
T, C, HS = 2048, 1024, 64
NSLICE = 4          # tq slices of 512
TS = T // NSLICE    # 512
CK = C // 128       # 8 c-chunks
NJ = T // 128       # 16 tk tiles of 128
SCALE = float(C) ** -0.5

# matmul dtype knob: float32 (exact, 4 cyc/row) vs float32r (1 cyc/row @ N>=512)
MM_DT = mybir.dt.bfloat16


DT = MM_DT  # dtype for every tensor feeding a matmul


def build_bass():
    nc = bacc.Bacc(None, target_bir_lowering=False)
    xq = nc.dram_tensor("xq", [NSLICE, C, TS], DT, kind="ExternalInput").ap()
    xk = nc.dram_tensor("xk", [NSLICE, C, TS], DT, kind="ExternalInput").ap()
    wq = nc.dram_tensor("wq", [C, HS], DT, kind="ExternalInput").ap()
    wkv = nc.dram_tensor("wkv", [C, 2 * HS], DT, kind="ExternalInput").ap()
    out = nc.dram_tensor("out", [T, HS], FP32, kind="ExternalOutput").ap()

    with tile.TileContext(nc) as tc:
        with (
            tc.tile_pool(name="singles", bufs=1) as singles,
            tc.tile_pool(name="xpool", bufs=2) as xpool,
            tc.tile_pool(name="projsb", bufs=1) as projsb,
            tc.tile_pool(name="pblk", bufs=2) as pblk,
            tc.tile_pool(name="fin", bufs=4) as fin,
            tc.tile_pool(name="pp_q", bufs=2, space="PSUM") as pp_q,
            tc.tile_pool(name="pp_kv", bufs=2, space="PSUM") as pp_kv,
            tc.tile_pool(name="pp_st", bufs=2, space="PSUM") as pp_st,
            tc.tile_pool(name="pp_o", bufs=1, space="PSUM") as pp_o,
            tc.tile_pool(name="pp_tr", bufs=1, space="PSUM") as pp_tr,
        ):
            # ---- one-time constants ----
            wq_sb = singles.tile([128, CK, HS], DT)
            nc.sync.dma_start(out=wq_sb, in_=wq.rearrange("(c p) h -> p c h", p=128))
            wkv_sb = singles.tile([128, CK, 2 * HS], DT)
            nc.sync.dma_start(out=wkv_sb, in_=wkv.rearrange("(c p) h -> p c h", p=128))

            ident = singles.tile([128, 128], FP32)
            make_identity(nc, ident)
            ident_dt = singles.tile([128, 128], DT)
            make_identity(nc, ident_dt)

            # diag masks: mask[m][x, y] = 1.0 if y - x >= 128*m else 0.0
            masks = singles.tile([128, 4, TS], DT)
            nc.gpsimd.memset(masks, 1.0)
            for m in range(4):
                nc.gpsimd.affine_select(
                    out=masks[:, m, :],
                    in_=masks[:, m, :],
                    compare_op=mybir.AluOpType.is_ge,
                    fill=0.0,
                    base=-128 * m,
                    pattern=[[1, TS]],
                    channel_multiplier=-1,
                )

            # persistent activations
            qT_sb = projsb.tile([64, T], DT)        # qT
            kvT_sb = projsb.tile([128, T], DT)      # rows 0:64 kT, 64:128 vT
            vT_sb = projsb.tile([64, T], DT)      # vT shifted to base 0
            v_sb = projsb.tile([128, NJ, HS + 1], DT)  # v natural + ones col
            ones_f32 = singles.tile([128, 1], FP32)
            nc.vector.memset(ones_f32, 1.0)
            nc.vector.tensor_copy(
                v_sb[:, :, HS : HS + 1], ones_f32.broadcast_to((128, NJ, 1))
            )

            for s in range(NSLICE):
                t0 = s * TS
                # ---- stream x slice ----
                xq_t = xpool.tile([128, CK, TS], DT, tag="xq")
                xk_t = xpool.tile([128, CK, TS], DT, tag="xk")
                xq_r = xq[s].rearrange("(c p) t -> p c t", p=128)
                xk_r = xk[s].rearrange("(c p) t -> p c t", p=128)
                h = CK // 2
                nc.sync.dma_start(out=xq_t[:, 0:h], in_=xq_r[:, 0:h])
                nc.sync.dma_start(out=xk_t[:, 0:h], in_=xk_r[:, 0:h])
                nc.sync.dma_start(out=xq_t[:, h:CK], in_=xq_r[:, h:CK])
                nc.sync.dma_start(out=xk_t[:, h:CK], in_=xk_r[:, h:CK])

                # ---- projections for this slice ----
                q_ps = pp_q.tile([64, TS], FP32)
                kv_ps = pp_kv.tile([128, TS], FP32)
                for ci in range(CK):
                    nc.tensor.matmul(
                        q_ps,
                        wq_sb[:, ci, :],
                        xq_t[:, ci, :],
                        start=(ci == 0),
                        stop=(ci == CK - 1),
                    )
                    nc.tensor.matmul(
                        kv_ps,
                        wkv_sb[:, ci, :],
                        xk_t[:, ci, :],
                        start=(ci == 0),
                        stop=(ci == CK - 1),
                    )
                nc.vector.tensor_copy(qT_sb[:, t0 : t0 + TS], q_ps)
                nc.vector.tensor_copy(kvT_sb[:, t0 : t0 + TS], kv_ps)
                # shift vT rows 64:128 -> base 0 (DMA can cross partitions)
                nc.sync.dma_start(
                    out=vT_sb[:, t0 : t0 + TS],
                    in_=kvT_sb[64:128, t0 : t0 + TS],
                )
                # v natural layout via PE transpose of [64,128] blocks
                for jj in range(4):
                    j = 4 * s + jj
                    v_ps = pp_tr.tile([128, HS], DT, tag="tr")
                    nc.tensor.transpose(
                        v_ps,
                        vT_sb[:, j * 128 : (j + 1) * 128],
                        ident_dt[0:64, 0:64],
                    )
                    nc.vector.tensor_copy(v_sb[:, j, 0:HS], v_ps)

                # ---- attention for tq-tile i = s ----
                i = s
                nj = 4 * i + 4
                p_all = pblk.tile([128, NJ, TS], DT, tag="p")
                for j in range(nj):
                    st_ps = pp_st.tile([128, TS], FP32)
                    nc.tensor.matmul(
                        st_ps,
                        kvT_sb[0:64, j * 128 : (j + 1) * 128],
                        qT_sb[:, t0 : t0 + TS],
                        start=True,
                        stop=True,
                    )
                    nc.scalar.activation(
                        out=p_all[:, j, :],
                        in_=st_ps,
                        func=mybir.ActivationFunctionType.Exp,
                        scale=SCALE,
                    )
                    if j >= 4 * i:
                        nc.vector.tensor_mul(
                            p_all[:, j, :], p_all[:, j, :], masks[:, j - 4 * i, :]
                        )
                o_ps = pp_o.tile([HS + 1, TS], FP32)
                for j in range(nj):
                    nc.tensor.matmul(
                        o_ps,
                        v_sb[:, j, :],
                        p_all[:, j, :],
                        start=(j == 0),
                        stop=(j == nj - 1),
                    )
                ot_sb = fin.tile([HS + 1, TS], FP32, tag="ot")
                nc.scalar.copy(ot_sb, o_ps)
                for u in range(4):
                    tr_ps = pp_tr.tile([128, HS + 1], FP32, tag="tr")
                    nc.tensor.transpose(
                        tr_ps,
                        ot_sb[:, u * 128 : (u + 1) * 128],
                        ident[0 : HS + 1, 0 : HS + 1],
                    )
                    rec = fin.tile([128, 1], FP32, tag="rec")
                    nc.vector.reciprocal(rec, tr_ps[:, HS : HS + 1])
                    ob = fin.tile([128, HS], FP32, tag="ob")
                    nc.vector.tensor_scalar_mul(ob, tr_ps[:, 0:HS], rec)
                    r0 = (4 * i + u) * 128
                    nc.sync.dma_start(out=out[r0 : r0 + 128, :], in_=ob)
    nc.compile()
    return nc


_NC_CACHE = {}


def _get_nc():
    key = str(MM_DT)
    if key not in _NC_CACHE:
        _NC_CACHE[key] = build_bass()
    return _NC_CACHE[key]


def kernel(x_q, x_kv, Wq, Wk, Wv, _trace=False):
    B = x_q.shape[0]
    assert B == 8 and x_q.shape == (8, T, C)
    hdt = ml_dtypes.bfloat16 if MM_DT == mybir.dt.bfloat16 else np.float32
    wkv = np.ascontiguousarray(np.concatenate([Wk, Wv], axis=1)).astype(hdt)
    wq = np.ascontiguousarray(Wq).astype(hdt)
    # [B, T, C] -> [B, C, T] -> [B, NSLICE, C, TS] (t-slice major, contiguous)
    xqT = np.ascontiguousarray(
        x_q.transpose(0, 2, 1).reshape(B, C, NSLICE, TS).transpose(0, 2, 1, 3)
    ).astype(hdt)
    xkT = np.ascontiguousarray(
        x_kv.transpose(0, 2, 1).reshape(B, C, NSLICE, TS).transpose(0, 2, 1, 3)
    ).astype(hdt)

    in_maps = [
        {"xq": xqT[b], "xk": xkT[b], "wq": wq, "wkv": wkv} for b in range(B)
    ]
    nc = _get_nc()
    res = run_bass_kernel_spmd(nc, in_maps, core_ids=list(range(B)), trace=_trace)
    out = np.stack([r["out"] for r in res.results])
    if _trace:
        kernel.last_result = res
    return out
